# revision 12
# baseline (speedup 1.0000x reference)
"""GNN encoder (2-layer GCN + inner-product decoder) on 8 TRN2 NeuronCores.

Sharding: nodes (rows of feat/z) are sharded across the 8 cores.
  phase 1: x1 = feat @ W1           row-sharded GEMM, then AllGather x1
  phase 2: y1 = A @ x1              per-core rows; gather x1[cols] via
                                    dma_gather, segment-sum via one-hot
                                    scatter-matrix matmuls (PSUM accumulate)
  phase 3: x2 = y1 @ W2, AllGather; z = A @ x2 (same spmm scheme); AllGather z
  phase 4: A_rec = sigmoid(z_shard @ z_full.T)   row-sharded GEMM, K=64

All matmuls in fp32 (the sigmoid decoder's logits have std ~2300 and the
near-threshold entries are precision-critical; bf16/tf32 anywhere pushes the
output rel-err to ~1e-2).
"""

import math
import numpy as np

import concourse.bass as bass
import concourse.bacc as bacc
import concourse.mybir as mybir
import concourse.tile as tile
from concourse import bass_utils

F32 = mybir.dt.float32
I16 = mybir.dt.int16


class Cfg:
    def __init__(self, N=20000, NC=8, D=512, H=256, O=64, TB=None):
        self.N = N          # total nodes
        self.NC = NC        # cores
        self.D = D          # input feature dim
        self.H = H          # hidden dim
        self.O = O          # output dim
        assert N % NC == 0
        self.R = N // NC    # rows per core
        self.NB = (self.R + 127) // 128   # row blocks per core
        self.RP = self.NB * 128           # padded rows per core
        self.KD = D // 128  # K tiles for GEMM1
        self.KH = H // 128  # K tiles for GEMM2
        self.TB = TB        # edge tiles per row block (set by preprocess)


def preprocess(feat, adj_rows, adj_cols, adj_vals, W1, W2, cfg):
    """Host-side: shard edges by destination row, group into row blocks,
    pad each (core, block) to a uniform TB tiles of 128 edges, and build
    the gather-index / local-row / edge-value arrays in the layouts the
    device kernel expects. Returns (in_maps, cfg)."""
    N, NC, R, NB = cfg.N, cfg.NC, cfg.R, cfg.NB
    rows = np.asarray(adj_rows).astype(np.int64)
    cols = np.asarray(adj_cols).astype(np.int64)
    vals = np.asarray(adj_vals).astype(np.float32)
    feat = np.asarray(feat, dtype=np.float32)
    W1 = np.asarray(W1, dtype=np.float32)
    W2 = np.asarray(W2, dtype=np.float32)

    core_of = rows // R
    er = rows - core_of * R           # row local to core
    blk = er // 128                   # row block within core
    lr = er - blk * 128               # row local to block

    # order edges by (core, block); within a block order is irrelevant
    order = np.lexsort((blk, core_of))
    core_s, blk_s, lr_s, col_s, val_s = (
        core_of[order], blk[order], lr[order], cols[order], vals[order])

    # per-(core, block) counts
    counts = np.zeros((NC, NB), dtype=np.int64)
    np.add.at(counts, (core_s, blk_s), 1)
    TB = int(math.ceil(counts.max() / 128.0))
    TB = max(TB, 1)
    cfg.TB = TB
    ntile_edges = TB * 128

    # split the sorted stream into per-(core, block) segments
    seg_end = np.cumsum(counts.reshape(-1))
    seg_start = np.concatenate([[0], seg_end[:-1]])

    iota = np.tile(np.arange(128, dtype=np.float32), (128, 1))
    ident = np.eye(128, dtype=np.float32)

    in_maps = []
    for c in range(NC):
        gidx = np.zeros((NB, ntile_edges), dtype=np.int16)
        lrow = np.zeros((NB, ntile_edges), dtype=np.float32)
        evals = np.zeros((NB, ntile_edges), dtype=np.float32)
        for b in range(NB):
            s, e = seg_start[c * NB + b], seg_end[c * NB + b]
            n = e - s
            gidx[b, :n] = col_s[s:e].astype(np.int16)
            lrow[b, :n] = lr_s[s:e].astype(np.float32)
            evals[b, :n] = val_s[s:e]
        # gather-index wrapped layout: idx i -> [i % 16, i // 16], replicated
        # over the 8 groups of 16 partitions
        gw = gidx.reshape(NB, ntile_edges // 16, 16).transpose(2, 0, 1)  # [16, NB, n/16]
        gw = gw.reshape(16, NB * (ntile_edges // 16))
        gidx_dev = np.tile(gw, (8, 1))  # [128, NB*n/16]
        # lrow/evals: per tile t, partition p holds edge t*128+p
        lrow_dev = lrow.reshape(NB, TB, 128).transpose(2, 0, 1).reshape(128, NB * TB).copy()
        evals_dev = evals.reshape(NB, TB, 128).transpose(2, 0, 1).reshape(128, NB * TB).copy()

        featp = np.zeros((cfg.RP, cfg.D), dtype=np.float32)
        featp[:R] = feat[c * R:(c + 1) * R]

        in_maps.append({
            "featp": featp,
            "w1": W1,
            "w2": W2,
            "gidx": gidx_dev,
            "lrow": lrow_dev,
            "evals": evals_dev,
            "iotat": iota,
            "ident": ident,
        })
    return in_maps, cfg


def build_bass(cfg, enable_asserts=False):
    N, NC, D, H, O = cfg.N, cfg.NC, cfg.D, cfg.H, cfg.O
    R, NB, RP, TB = cfg.R, cfg.NB, cfg.RP, cfg.TB
    KD, KH = cfg.KD, cfg.KH
    IE = TB * 128            # edges per block (padded)
    IW = IE // 16            # gidx columns per block

    nc = bacc.Bacc("TRN2", target_bir_lowering=False, debug=False,
                   enable_asserts=enable_asserts, num_devices=NC)

    featp = nc.dram_tensor("featp", [RP, D], F32, kind="ExternalInput").ap()
    w1d = nc.dram_tensor("w1", [D, H], F32, kind="ExternalInput").ap()
    w2d = nc.dram_tensor("w2", [H, O], F32, kind="ExternalInput").ap()
    gidxd = nc.dram_tensor("gidx", [128, NB * IW], I16, kind="ExternalInput").ap()
    lrowd = nc.dram_tensor("lrow", [128, NB * TB], F32, kind="ExternalInput").ap()
    evalsd = nc.dram_tensor("evals", [128, NB * TB], F32, kind="ExternalInput").ap()
    iotad = nc.dram_tensor("iotat", [128, 128], F32, kind="ExternalInput").ap()
    identd = nc.dram_tensor("ident", [128, 128], F32, kind="ExternalInput").ap()

    out_z = nc.dram_tensor("out_z", [R, O], F32, kind="ExternalOutput").ap()
    out_a = nc.dram_tensor("out_a", [R, N], F32, kind="ExternalOutput").ap()

    rg = [list(range(NC))]
    shared = "Shared" if NC > 4 else "Local"

    with tile.TileContext(nc) as tc:
        with tc.tile_pool(name="const", bufs=1) as cpool, \
             tc.tile_pool(name="dram", bufs=1, space="DRAM") as dram:

            # ---- constants / small inputs -------------------------------
            iota_t = cpool.tile([128, 128], F32)
            nc.sync.dma_start(iota_t[:, :], iotad)
            ident_t = cpool.tile([128, 128], F32)
            nc.sync.dma_start(ident_t[:, :], identd)
            w1_t = cpool.tile([128, KD * H], F32)
            nc.sync.dma_start(
                w1_t[:, :].rearrange("p (t h) -> p t h", h=H),
                w1d.rearrange("(t p) h -> p t h", p=128))
            w2_t = cpool.tile([128, KH * O], F32)
            nc.sync.dma_start(
                w2_t[:, :].rearrange("p (t h) -> p t h", h=O),
                w2d.rearrange("(t p) h -> p t h", p=128))
            gidx_t = cpool.tile([128, NB * IW], I16)
            nc.sync.dma_start(gidx_t[:, :], gidxd)
            lrow_t = cpool.tile([128, NB * TB], F32)
            nc.sync.dma_start(lrow_t[:, :], lrowd)
            evals_t = cpool.tile([128, NB * TB], F32)
            nc.sync.dma_start(evals_t[:, :], evalsd)

            x1_bounce = dram.tile([R, H], F32)
            x1_full = dram.tile([N, H], F32, addr_space=shared)

            # ---- phase 1: x1 = feat @ W1, AllGather ---------------------
            with tc.tile_pool(name="p1", bufs=3) as pool, \
                 tc.tile_pool(name="p1ps", bufs=2, space="PSUM") as psT, \
                 tc.tile_pool(name="p1ps2", bufs=2, space="PSUM") as ps1:
                for nt in range(NB):
                    ft = pool.tile([128, D], F32, tag="ft")
                    nc.sync.dma_start(ft[:, :], featp[nt * 128:(nt + 1) * 128, :])
                    x1ps = ps1.tile([128, H], F32, tag="x1ps")
                    for kt in range(KD):
                        tps = psT.tile([128, 128], F32, tag="tps")
                        nc.tensor.transpose(tps[:, :], ft[:, kt * 128:(kt + 1) * 128],
                                            ident_t[:, :])
                        fT = pool.tile([128, 128], F32, tag="fT")
                        nc.vector.tensor_copy(fT[:, :], tps[:, :])
                        nc.tensor.matmul(x1ps[:, :], fT[:, :],
                                         w1_t[:, kt * H:(kt + 1) * H],
                                         start=(kt == 0), stop=(kt == KD - 1))
                    x1sb = pool.tile([128, H], F32, tag="x1sb")
                    nc.vector.tensor_copy(x1sb[:, :], x1ps[:, :])
                    v = min(128, R - nt * 128)
                    nc.sync.dma_start(x1_bounce[nt * 128:nt * 128 + v, :], x1sb[:v, :])

            nc.gpsimd.collective_compute(
                "AllGather", mybir.AluOpType.bypass, replica_groups=rg,
                ins=[x1_bounce.opt()], outs=[x1_full.opt()])

            # ---- phase 2: y1 = A @ x1 (spmm via one-hot matmuls) --------
            y1cm = tc.tile_pool(name="y1p", bufs=1)
            y1pool = y1cm.__enter__()
            y1 = y1pool.tile([128, NB * H], F32)  # row-block b at [:, b*H:(b+1)*H]
            with tc.tile_pool(name="p2g", bufs=2) as gpool, \
                 tc.tile_pool(name="p2s", bufs=4) as spool, \
                 tc.tile_pool(name="p2ps", bufs=2, space="PSUM") as ps2:
                for b in range(NB):
                    g1 = gpool.tile([128, TB * H], F32, tag="g1")
                    nc.gpsimd.dma_gather(
                        out_ap=g1[:, :].rearrange("p (t h) -> p t h", h=H),
                        in_ap=x1_full[:, :],
                        idxs_ap=gidx_t[:, b * IW:(b + 1) * IW],
                        num_idxs=IE, num_idxs_reg=IE, elem_size=H,
                        single_packet=False)
                    yps = ps2.tile([128, H], F32, tag="yps")
                    for t in range(TB):
                        g = b * TB + t
                        S = spool.tile([128, 128], F32, tag="S")
                        nc.vector.tensor_scalar(
                            S[:, :], iota_t[:, :], lrow_t[:, g:g + 1],
                            evals_t[:, g:g + 1],
                            mybir.AluOpType.is_equal, mybir.AluOpType.mult)
                        nc.tensor.matmul(yps[:, :], S[:, :],
                                         g1[:, t * H:(t + 1) * H],
                                         start=(t == 0), stop=(t == TB - 1))
                    nc.vector.tensor_copy(y1[:, b * H:(b + 1) * H], yps[:, :])

            # ---- phase 3a: x2 = y1 @ W2, AllGather ----------------------
            x2_bounce = dram.tile([R, O], F32)
            x2_full = dram.tile([N, O], F32, addr_space=shared)
            with tc.tile_pool(name="p3", bufs=3) as pool, \
                 tc.tile_pool(name="p3ps", bufs=2, space="PSUM") as psT, \
                 tc.tile_pool(name="p3ps2", bufs=2, space="PSUM") as ps3:
                for nt in range(NB):
                    x2ps = ps3.tile([128, O], F32, tag="x2ps")
                    for kt in range(KH):
                        tps = psT.tile([128, 128], F32, tag="tps3")
                        nc.tensor.transpose(
                            tps[:, :], y1[:, nt * H + kt * 128: nt * H + (kt + 1) * 128],
                            ident_t[:, :])
                        yT = pool.tile([128, 128], F32, tag="yT")
                        nc.vector.tensor_copy(yT[:, :], tps[:, :])
                        nc.tensor.matmul(x2ps[:, :], yT[:, :],
                                         w2_t[:, kt * O:(kt + 1) * O],
                                         start=(kt == 0), stop=(kt == KH - 1))
                    x2sb = pool.tile([128, O], F32, tag="x2sb")
                    nc.vector.tensor_copy(x2sb[:, :], x2ps[:, :])
                    v = min(128, R - nt * 128)
                    nc.sync.dma_start(x2_bounce[nt * 128:nt * 128 + v, :], x2sb[:v, :])

            nc.gpsimd.collective_compute(
                "AllGather", mybir.AluOpType.bypass, replica_groups=rg,
                ins=[x2_bounce.opt()], outs=[x2_full.opt()])
            y1cm.__exit__(None, None, None)

            # zT tiles outlive zsh (phase 4) — enter their pool first (LIFO)
            ztcm = tc.tile_pool(name="ztp", bufs=1)
            ztpool = ztcm.__enter__()
            zTsh = ztpool.tile([O, RP], F32)
            zTall = ztpool.tile([O, N], F32)

            # ---- phase 3b: z = A @ x2 (spmm) ----------------------------
            zcm = tc.tile_pool(name="zp", bufs=1)
            zpool = zcm.__enter__()
            zsh = zpool.tile([128, NB * O], F32)
            z_bounce = dram.tile([R, O], F32)
            z_full = dram.tile([N, O], F32, addr_space=shared)
            with tc.tile_pool(name="p3b", bufs=2) as gpool, \
                 tc.tile_pool(name="p3bs", bufs=4) as spool, \
                 tc.tile_pool(name="p3bps", bufs=2, space="PSUM") as psz:
                for b in range(NB):
                    g2 = gpool.tile([128, TB * O], F32, tag="g2")
                    nc.gpsimd.dma_gather(
                        out_ap=g2[:, :].rearrange("p (t h) -> p t h", h=O),
                        in_ap=x2_full[:, :],
                        idxs_ap=gidx_t[:, b * IW:(b + 1) * IW],
                        num_idxs=IE, num_idxs_reg=IE, elem_size=O,
                        single_packet=False)
                    zps = psz.tile([128, O], F32, tag="zps")
                    for t in range(TB):
                        g = b * TB + t
                        S = spool.tile([128, 128], F32, tag="S2")
                        nc.vector.tensor_scalar(
                            S[:, :], iota_t[:, :], lrow_t[:, g:g + 1],
                            evals_t[:, g:g + 1],
                            mybir.AluOpType.is_equal, mybir.AluOpType.mult)
                        nc.tensor.matmul(zps[:, :], S[:, :],
                                         g2[:, t * O:(t + 1) * O],
                                         start=(t == 0), stop=(t == TB - 1))
                    nc.vector.tensor_copy(zsh[:, b * O:(b + 1) * O], zps[:, :])
                    v = min(128, R - b * 128)
                    nc.sync.dma_start(out_z[b * 128:b * 128 + v, :],
                                      zsh[:v, b * O:(b + 1) * O])
                    nc.sync.dma_start(z_bounce[b * 128:b * 128 + v, :],
                                      zsh[:v, b * O:(b + 1) * O])

            nc.gpsimd.collective_compute(
                "AllGather", mybir.AluOpType.bypass, replica_groups=rg,
                ins=[z_bounce.opt()], outs=[z_full.opt()])

            # ---- zT for the decoder ------------------------------------
            # zT_shard [O, RP] from local z tiles (pre-AG, no core-dependent
            # addressing); zT_all [O, N] from the AllGathered z.
            with tc.tile_pool(name="pzt", bufs=1) as pool, \
                 tc.tile_pool(name="pztps", bufs=4, space="PSUM") as psT:
                for b in range(NB):
                    tps = psT.tile([O, 128], F32, tag="tpzl")
                    nc.tensor.transpose(tps[:, :], zsh[:, b * O:(b + 1) * O],
                                        ident_t[:, :])
                    nc.vector.tensor_copy(zTsh[:, b * 128:(b + 1) * 128], tps[:, :])

                NT = N // 128
                rem = N - NT * 128
                zf = pool.tile([128, NT * O], F32, tag="zf")
                nc.sync.dma_start(
                    zf[:, :].rearrange("p (t h) -> p t h", h=O),
                    z_full[:NT * 128, :].rearrange("(t p) h -> p t h", p=128))
                for t in range(NT):
                    tps = psT.tile([O, 128], F32, tag="tpza")
                    nc.tensor.transpose(tps[:, :], zf[:, t * O:(t + 1) * O],
                                        ident_t[:, :])
                    nc.vector.tensor_copy(zTall[:, t * 128:(t + 1) * 128], tps[:, :])
                if rem:
                    zfr = pool.tile([128, O], F32, tag="zfr")
                    nc.sync.dma_start(zfr[:rem, :], z_full[NT * 128:, :])
                    tps = psT.tile([O, 128], F32, tag="tpza")
                    nc.tensor.transpose(tps[:, :rem], zfr[:rem, :], ident_t[:rem, :rem])
                    nc.vector.tensor_copy(zTall[:, NT * 128:], tps[:, :rem])
            zcm.__exit__(None, None, None)

            # ---- phase 4: A_rec = sigmoid(z_shard @ z_full.T) -----------
            CG = 2048                      # output staging columns per DMA
            NG = (N + CG - 1) // CG
            with tc.tile_pool(name="p4o", bufs=3) as opool, \
                 tc.tile_pool(name="p4ps", bufs=4, space="PSUM") as ps4:
                for rt in range(NB):
                    v = min(128, R - rt * 128)
                    lhsT = zTsh[:, rt * 128:(rt + 1) * 128]
                    for gcol in range(NG):
                        c0 = gcol * CG
                        cw = min(CG, N - c0)
                        ab = opool.tile([128, CG], F32, tag="ab")
                        for j in range(0, cw, 512):
                            w = min(512, cw - j)
                            aps = ps4.tile([128, 512], F32, tag="aps")
                            nc.tensor.matmul(aps[:, :w], lhsT, zTall[:, c0 + j:c0 + j + w],
                                             start=True, stop=True)
                            nc.scalar.activation(ab[:, j:j + w], aps[:, :w],
                                                 mybir.ActivationFunctionType.Sigmoid)
                        nc.sync.dma_start(out_a[rt * 128:rt * 128 + v, c0:c0 + cw],
                                          ab[:v, :cw])
            ztcm.__exit__(None, None, None)

    nc.compile()
    return nc


_CACHE = {}


def kernel(feat, adj_rows, adj_cols, adj_vals, W1, W2):
    cfg = Cfg()
    in_maps, cfg = preprocess(feat, adj_rows, adj_cols, adj_vals, W1, W2, cfg)
    key = ("k", cfg.N, cfg.NC, cfg.TB)
    if key not in _CACHE:
        _CACHE[key] = build_bass(cfg)
    nc = _CACHE[key]
    res = bass_utils.run_bass_kernel_spmd(
        nc, in_maps, core_ids=list(range(cfg.NC)))
    z = np.concatenate([res.results[c]["out_z"] for c in range(cfg.NC)], axis=0)
    A = np.concatenate([res.results[c]["out_a"] for c in range(cfg.NC)], axis=0)
    return z, A
